# revision 69
# baseline (speedup 1.0000x reference)
"""BigBird block kernel for 8 Trainium2 NeuronCores (v3).

Sharding: core p in 0..7 -> batch b = p//2, head-half j = p%2 (heads 4j..4j+3).
Each core: LN1 over its full batch-row, q/k/v for its 4 heads over the full
sequence, block-sparse attention for all 64 query blocks (its heads), then a
partial output projection.  The projection partials are pairwise
ReduceScattered in 4 token chunks, each issued as soon as its attention
blocks finish so the collectives overlap the remaining attention; FFN chunks
are interleaved into the schedule one RS chunk behind.  Residual + LN2 + FFN
run on the core's own 2048-token half; the host reassembles 8 halves.

v3 changes vs v2:
- Scores are computed TRANSPOSED (sT[j, q] = kT_block.T @ qd): the exp
  output pT feeds the AV matmuls directly, eliminating all per-unit PE
  transposes and the pt_sb copies.
- Softmax row sums come from a ones column baked into the v tiles (group
  width 258 = 2 heads x (128 feat + 1)); normalization divides the AV
  output rows as before.
- The FFN runs in fp8(e4m3) DoubleRow perf mode (weights pre-scaled by 64
  host-side, rescaled in the activation), halving PE time there.
- Phase-1 engine rebalance: transpose writebacks go to GpSimd, v copies
  alternate Vector/GpSimd.
All other matmuls bf16 with fp32 PSUM accumulation.
"""

import numpy as np
import ml_dtypes

import concourse.bass as bass
import concourse.bacc as bacc
import concourse.tile as tile
from concourse import mybir
from concourse import masks
from concourse import bass_utils

# ---- problem constants (hardcoded; must match the reference) ----
B, S, D = 4, 4096, 512
H, DH = 8, 64
M = 2048
BS = 64
NB = S // BS            # 64 blocks
R = 3
K = 8
SEED = 0
LN_EPS = 1e-6

HL = 4                  # local heads per core
F = HL * DH             # 256 local qkv features
TH = S // 2             # 2048 tokens per core half
NEG_FILL = -30.0        # exp(-30) ~ 1e-13: masked-slot fill before exp
VG = 258                # v group width: 2 hp x (128 features + ones col)
W8 = 64.0               # host-side fp8 weight pre-scale for the FFN

FP32 = mybir.dt.float32
BF16 = mybir.dt.bfloat16
F8 = mybir.dt.float8e4
AF = mybir.ActivationFunctionType
ALU = mybir.AluOpType
AX = mybir.AxisListType
DR = mybir.MatmulPerfMode.DoubleRow


def _block_indices():
    """Static BigBird connectivity (identical to the reference)."""
    rng = np.random.RandomState(SEED)
    idx = np.zeros((NB - 2, K), np.int32)
    msk = np.zeros((NB - 2, K), bool)
    for row, i in enumerate(range(1, NB - 1)):
        fixed = sorted({0, NB - 1, i - 1, i, i + 1})
        allowed = [b for b in range(NB) if b not in fixed]
        rand = rng.choice(allowed, size=R, replace=False).tolist()
        blocks = fixed + rand
        idx[row, :len(blocks)] = blocks
        msk[row, :len(blocks)] = True
    return idx, msk


IDX, KMASK = _block_indices()


def _group_pairs(gsz=4):
    """Per gsz-block half-group: the (row, chunk) slots whose key/value
    blocks are non-adjacent ('pair') or padded ('single'), each gathered
    into a [128, VG] v tile (block lo -> rows 0-63, hi -> rows 64-127;
    singles duplicate their block, the dup side sees only exp(NEG_FILL))
    and, per hp, a [128, 128] k tile for one contiguous score matmul
    (singles keep the quadrant score path + NEG_FILL memset).
    Returns {grp: [(i, ci, b0, b1, kind), ...]} and {(i, ci): slot}."""
    groups, slots = {}, {}
    for i in range(1, NB - 1):
        for ci, (kind, bl) in enumerate(_row_slots(i)):
            if kind == "pair" or kind == "single":
                b0, b1 = bl
                if b1 is None:
                    b1 = b0
                g = i // gsz
                lst = groups.setdefault(g, [])
                slots[(i, ci)] = len(lst)
                lst.append((i, ci, b0, b1, kind))
    return groups, slots


def _row_slots(i):
    """Slot layout for interior block i: list of (kind, blocks) chunk pairs.

    Each chunk covers 2 of the 8 score-column slots.  kind: 'nat' = natural
    v pair (even,even+1), 'glob' = (0,63) via v_glob, 'pair' = 2 unrelated
    singles, 'single' = one block (+pad).  Returns a list of 4 entries, each
    (kind, [b0, b1]) with b1 None for a padded single.
    """
    row = i - 1
    bs = sorted(int(IDX[row, s]) for s in range(K) if KMASK[row, s])
    rest = [b for b in bs if b not in (0, NB - 1)]
    nat, singles = [], []
    j = 0
    while j < len(rest):
        if j + 1 < len(rest) and rest[j] % 2 == 0 and rest[j + 1] == rest[j] + 1:
            nat.append((rest[j], rest[j + 1]))
            j += 2
        else:
            singles.append(rest[j])
            j += 1
    chunks = [("glob", [0, NB - 1])]
    for p in nat:
        chunks.append(("nat", list(p)))
    j = 0
    while j < len(singles):
        if j + 1 < len(singles):
            chunks.append(("pair", [singles[j], singles[j + 1]]))
            j += 2
        else:
            chunks.append(("single", [singles[j], None]))
            j += 1
    assert len(chunks) <= 4
    while len(chunks) < 4:
        chunks.append(("empty", [None, None]))
    return chunks


def build_kernel(rs_nchunks=4, no_coll=False, ffn_fp8=False, ffn_slack=2,
                 psa_bufs=4, pp_bufs=3):
    nc = bacc.Bacc(
        "TRN2",
        target_bir_lowering=False,
        debug=False,
        num_devices=8,
    )

    wdt = F8 if ffn_fp8 else BF16

    # ---- DRAM I/O ----
    x_full = nc.dram_tensor("x_full", [S, D], BF16, kind="ExternalInput").ap()
    x_own = nc.dram_tensor("x_own", [TH, D], BF16, kind="ExternalInput").ap()
    wq_d = nc.dram_tensor("wq", [D, F], BF16, kind="ExternalInput").ap()
    wk_d = nc.dram_tensor("wk", [D, F], BF16, kind="ExternalInput").ap()
    wv_d = nc.dram_tensor("wv", [D, F], BF16, kind="ExternalInput").ap()
    wo_d = nc.dram_tensor("wo", [F, D], BF16, kind="ExternalInput").ap()
    w1_d = nc.dram_tensor("w1", [D, M], wdt, kind="ExternalInput").ap()
    w2_d = nc.dram_tensor("w2", [M, D], wdt, kind="ExternalInput").ap()
    qb_d = nc.dram_tensor("qb", [F], FP32, kind="ExternalInput").ap()
    kb_d = nc.dram_tensor("kb", [F], FP32, kind="ExternalInput").ap()
    b1_d = nc.dram_tensor("b1", [M], FP32, kind="ExternalInput").ap()
    b2_d = nc.dram_tensor("b2", [D], FP32, kind="ExternalInput").ap()
    out_d = nc.dram_tensor("out", [TH, D], FP32, kind="ExternalOutput").ap()

    rs_in = nc.dram_tensor("rs_in", [S, D], BF16).ap()
    rs_out = nc.dram_tensor("rs_out", [TH, D], BF16).ap()

    with tile.TileContext(nc) as tc:
        with (
            tc.tile_pool(name="const", bufs=1) as const,
            tc.tile_pool(name="persist", bufs=1) as persist,
            tc.tile_pool(name="xin", bufs=3) as xin,
            tc.tile_pool(name="xo_pool", bufs=2) as xo_pool,
            tc.tile_pool(name="stats", bufs=6) as stats,
            tc.tile_pool(name="tnat", bufs=6) as tnat_pool,
            tc.tile_pool(name="vpair", bufs=2) as vpair_pool,
            tc.tile_pool(name="psA", bufs=psa_bufs, space="PSUM") as psA,
            tc.tile_pool(name="psT", bufs=2, space="PSUM") as psT,
            tc.tile_pool(name="psO", bufs=2, space="PSUM") as psO,
            tc.tile_pool(name="p_pool", bufs=pp_bufs) as p_pool,
            tc.tile_pool(name="ostream", bufs=2) as ostream,
            tc.tile_pool(name="oin_pool", bufs=2) as oin_pool,
            tc.tile_pool(name="qd_pool", bufs=2) as qd_pool,
            tc.tile_pool(name="ffn", bufs=2) as ffn_pool,
            tc.tile_pool(name="hpool", bufs=1) as h_pool,
        ):
            # ---------------- constants ----------------
            ident = const.tile([128, 128], BF16, name="ident", tag="ident")
            masks.make_identity(nc, ident[:])

            eps_t = const.tile([128, 1], FP32, name="eps", tag="eps")
            nc.vector.memset(eps_t[:], LN_EPS)

            def load_cols(dram_ap, n, tag):
                t = const.tile([128, n // 128], FP32, name=tag, tag=tag)
                nc.sync.dma_start(out=t[:], in_=dram_ap.rearrange("(t p) -> p t", p=128))
                return t

            qb_sb = load_cols(qb_d, F, "qb")
            kb_sb = load_cols(kb_d, F, "kb")
            b1_sb = load_cols(b1_d, M, "b1")

            b2_rep = const.tile([128, D], FP32, name="b2rep", tag="b2rep")
            nc.sync.dma_start(
                out=b2_rep[:],
                in_=bass.AP(tensor=b2_d.tensor, offset=b2_d.offset,
                            ap=[[0, 128]] + b2_d.ap),
            )

            # weights: one SBUF tile per matrix, d-tile-major columns, 1 DMA each
            def load_w(dram_ap, rows, cols, tag, dt=BF16):
                nt = rows // 128
                t = const.tile([128, nt * cols], dt, name=tag, tag=tag)
                nc.sync.dma_start(
                    out=t[:].rearrange("p (t f) -> p t f", t=nt),
                    in_=dram_ap.rearrange("(t p) f -> p t f", p=128))
                return t

            # early weights only: q/k/v are needed from the first qk chunk.
            # wo/w1/w2 (5 MB) are DMA'd after phase 1 so they don't queue
            # ahead of the x tiles on the HBM path.
            wq_all = load_w(wq_d, D, F, "wqall")     # [:, d*F + f]
            wk_all = load_w(wk_d, D, F, "wkall")
            wv_all = load_w(wv_d, D, F, "wvall")
            # persistent activations
            big_all = persist.tile([128, 4 * S], BF16, name="bigall", tag="bigall")
            tT = [big_all[:, d * S:(d + 1) * S] for d in range(4)]
            qT = [persist.tile([128, S], BF16, name=f"qT{c}", tag=f"qT{c}")
                  for c in range(2)]
            kT = [persist.tile([128, S], BF16, name=f"kT{c}", tag=f"kT{c}")
                  for c in range(2)]
            v_all = persist.tile([128, 32 * VG], BF16, name="vall", tag="vall")
            group_pairs, pair_slot = _group_pairs(8)
            npair_max = max(len(v) for v in group_pairs.values())
            # o_pair reuses big_all cols [0, 2S); oT (local heads) in [2S, 4S)
            o_pair = [big_all[:, c * S:(c + 1) * S] for c in range(2)]
            oT = [big_all[:, (2 + c) * S:(3 + c) * S] for c in range(2)]
            v_glob = persist.tile([128, VG], BF16, name="vglob", tag="vglob")
            kg = [persist.tile([128, 128], BF16, name=f"kg{c}", tag=f"kg{c}")
                  for c in range(2)]

            vv = v_all[:].rearrange("p (g h c) -> p g h c", g=32, h=2)
            # ones columns (rowsum accumulators) in v_all; v_sh copies them
            for hp in range(2):
                nc.vector.memset(vv[:, :, hp, 128:129], 1.0)

            def vcols(t, g, hp):
                return t[:, g * VG + hp * 129:g * VG + hp * 129 + 129]

            def cp(out, in_):
                nc.vector.tensor_copy(out=out, in_=in_)

            # ---------------- phase 2 helpers: q/k/v projections ----------------
            def emit_qk_chunk(tck):
                for w_sb, bias_sb, dstT in ((wq_all, qb_sb, qT), (wk_all, kb_sb, kT)):
                    for ft in range(2):
                        ps = psA.tile([128, 512], FP32, name="psA", tag="psA")
                        for d in range(4):
                            nc.tensor.matmul(
                                ps[:], w_sb[:, d * F + ft * 128:d * F + (ft + 1) * 128],
                                tT[d][:, tck * 512:(tck + 1) * 512],
                                start=(d == 0), stop=(d == 3),
                            )
                        if ft == 0:
                            nc.scalar.activation(
                                out=dstT[ft][:, tck * 512:(tck + 1) * 512],
                                in_=ps[:], func=AF.Identity,
                                bias=bias_sb[:, ft:ft + 1], scale=1.0,
                            )
                        else:
                            nc.vector.tensor_scalar_add(
                                out=dstT[ft][:, tck * 512:(tck + 1) * 512],
                                in0=ps[:], scalar1=bias_sb[:, ft:ft + 1],
                            )

            # v bias is folded into x_own host-side (softmax weights sum to 1,
            # so o = (sum p v)/r + vb exactly; vb@wo is a constant per token)
            def emit_v_tile(tt):
                ps = psA.tile([128, 512], FP32, name="psA", tag="psA")
                for d in range(4):
                    nc.tensor.matmul(
                        ps[:, 0:F], tT[d][:, tt * 128:(tt + 1) * 128],
                        wv_all[:, d * F:(d + 1) * F],
                        start=(d == 0), stop=(d == 3),
                    )
                dst = vv[:, tt, :, 0:128]
                src = ps[:, 0:F].rearrange("p (h c) -> p h c", h=2)
                if tt % 2 == 0:
                    cp(dst, src)
                else:
                    nc.scalar.copy(out=dst, in_=src)

            # ---------------- phase 1: LN1 + transpose ----------------
            def ln1_tile(xt, tt, j):
                st = stats.tile([128, 6], FP32, name="bst", tag="bst")
                nc.vector.bn_stats(out=st[:], in_=xt)
                mv = stats.tile([128, 2], FP32, name="mv", tag="mv")
                nc.vector.bn_aggr(out=mv[:], in_=st[:])
                std = stats.tile([128, 1], FP32, name="std", tag="std")
                nc.scalar.activation(out=std[:], in_=mv[:, 1:2], func=AF.Sqrt,
                                     bias=eps_t[:], scale=1.0)
                inv = stats.tile([128, 1], FP32, name="inv", tag="inv")
                nc.vector.reciprocal(out=inv[:], in_=std[:])
                tn = tnat_pool.tile([128, D], BF16, name="tn", tag="tn")
                if j == 0:
                    nmi = stats.tile([128, 1], FP32, name="nmi", tag="nmi")
                    nc.vector.tensor_scalar(out=nmi[:], in0=mv[:, 0:1],
                                            scalar1=inv[:], scalar2=-1.0,
                                            op0=ALU.mult, op1=ALU.mult)
                    nc.scalar.activation(out=tn[:], in_=xt, func=AF.Identity,
                                         bias=nmi[:], scale=inv[:])
                else:
                    nc.vector.tensor_scalar(out=tn[:], in0=xt,
                                            scalar1=mv[:, 0:1], scalar2=inv[:],
                                            op0=ALU.subtract, op1=ALU.mult)
                pst = psT.tile([128, 512], BF16, name="psT", tag="psT")
                for c in range(4):
                    nc.tensor.transpose(pst[:, c * 128:(c + 1) * 128],
                                        tn[:, c * 128:(c + 1) * 128], ident[:])
                dst = big_all[:].rearrange("p (c s) -> p c s", c=4)[
                    :, :, tt * 128:(tt + 1) * 128]
                if j == 0:
                    cp(dst, pst[:].rearrange("p (c s) -> p c s", c=4))
                else:
                    nc.scalar.copy(
                        out=dst, in_=pst[:].rearrange("p (c s) -> p c s", c=4))

            with nc.named_scope("ph1"):
                for tq in range(16):
                    xt4 = xin.tile([128, 2, D], BF16, name="xt4", tag="xt4")
                    nc.sync.dma_start(
                        out=xt4[:],
                        in_=x_full.rearrange("(a p) d -> p a d", p=128)[:, tq * 2:(tq + 1) * 2, :])
                    for j in range(2):
                        ln1_tile(xt4[:, j, :], tq * 2 + j, j)
                    if tq % 2 == 1:
                        k4 = tq // 2
                        emit_qk_chunk(k4)
                        for vt in range(4 * k4, 4 * k4 + 4):
                            emit_v_tile(vt)

                # v_glob: block 0 (top of group 0) / block 63 (bottom of grp 31)
                cp(v_glob[0:64, :], v_all[0:64, 0:VG])
                cp(v_glob[64:128, :], v_all[64:128, 31 * VG:32 * VG])
                # kg: global key pair (block 0 | block 63) gathered adjacent so
                # the glob score chunk is one contiguous stationary matmul
                for hp in range(2):
                    cp(kg[hp][:, 0:64], kT[hp][:, 0:64])
                    cp(kg[hp][:, 64:128], kT[hp][:, 63 * 64:64 * 64])

            # late weights (first used >=150us in): wo at the first outproj,
            # w1/w2/b1/b2 at the first ffn chunk
            wo_all = load_w(wo_d, F, D, "woall")     # [:, c*D + e], c = f-chunk
            w1_all = load_w(w1_d, D, M, "w1all", wdt)   # [:, d*M + m]
            w2_all = load_w(w2_d, M, D, "w2all", wdt)   # [:, m*D + e]
            w1v = w1_all[:].rearrange("p (d m) -> p d m", d=4)
            w2v = w2_all[:].rearrange("p (m e) -> p m e", m=16)

            # block-diagonal q tiles, built per 8-block group on GpSimd.
            # Layout: [128, 2 hp x 8 blocks x 128].  Used as the MOVING
            # operand of the transposed score matmuls.
            qd_gen = [0]

            def build_qd_grp(blk0):
                eng = nc.gpsimd
                qd = qd_pool.tile([128, 2 * 8 * 128], BF16, name="qd", tag="qd")
                # zeros persist across pool rotation; same-engine program
                # order makes gen>=2 reads safe without re-zeroing
                if qd_gen[0] < 2:
                    eng.memset(qd[:], 0.0)
                qd_gen[0] += 1
                for hp in range(2):
                    for ir in range(8):
                        i = blk0 + ir
                        for h2 in range(2):
                            pr = slice(h2 * 64, h2 * 64 + 64)
                            c0 = (hp * 8 + ir) * 128 + h2 * 64
                            eng.tensor_copy(
                                out=qd[pr, c0:c0 + 64],
                                in_=qT[hp][pr, i * 64:(i + 1) * 64],
                            )
                return qd

            def qd_slice(qd, blk0, i, hp):
                c0 = (hp * 8 + (i - blk0)) * 128
                return qd[:, c0:c0 + 128]

            # per-group gathered v tiles for 'pair'/'single' chunks: block lo
            # -> rows 0-63, hi -> rows 64-127 (both hp slices + ones cols),
            # so every AV matmul is one full-128 chain link.
            def build_gathers(grp):
                pairs = group_pairs.get(grp, [])
                vp = vpair_pool.tile([128, npair_max * VG], BF16,
                                     name="vp", tag="vp")
                for slot, (i, ci, b0, b1, kind) in enumerate(pairs):
                    for sp, blk in enumerate((b0, b1)):
                        nc.sync.dma_start(
                            out=vp[sp * 64:sp * 64 + 64,
                                   slot * VG:(slot + 1) * VG],
                            in_=v_all[(blk % 2) * 64:(blk % 2) * 64 + 64,
                                      (blk // 2) * VG:(blk // 2 + 1) * VG])
                return vp

            # ---------------- global block unit (full attention) ----------------
            def emit_global(g, hp, qd):
                ps_og = psO.tile([128, 129], FP32, name="psO", tag="psO")
                for kc in range(8):
                    ps_s = psA.tile([128, 512], FP32, name="psA", tag="psA")
                    for c in range(4):
                        j0 = (kc * 4 + c) * 128
                        nc.tensor.matmul(
                            ps_s[:, c * 128:(c + 1) * 128],
                            kT[hp][:, j0:j0 + 128], qd,
                            start=True, stop=True,
                        )
                    p_sb = p_pool.tile([128, 512], BF16, name="p_g", tag="p")
                    nc.scalar.activation(out=p_sb[:], in_=ps_s[:], func=AF.Exp)
                    for c in range(4):
                        sc = kc * 4 + c
                        nc.tensor.matmul(
                            ps_og[:], p_sb[:, c * 128:(c + 1) * 128],
                            vcols(v_all, sc, hp),
                            start=(sc == 0), stop=(sc == 31),
                        )
                pinv = stats.tile([128, 1], FP32, name="pinv", tag="pinv")
                nc.vector.reciprocal(out=pinv[:], in_=ps_og[:, 128:129])
                for h2 in range(2):
                    pr = slice(h2 * 64, h2 * 64 + 64)
                    dst = o_pair[hp][pr, g * 64:(g + 1) * 64]
                    nc.vector.tensor_scalar_mul(
                        out=dst, in0=ps_og[pr, pr], scalar1=pinv[pr],
                    )

            # ---------------- interior block-sparse attention ----------------
            def attn_stage1(i, hp, qd):
                """Transposed scores sT[j, q] for both heads in one matmul per
                2-block chunk (stationary kT cols, moving block-diag qd)."""
                chunks = _row_slots(i)
                ps_s = psA.tile([128, 512], FP32, name="psA", tag="psA")
                for ci, (kind, bl) in enumerate(chunks):
                    col0 = ci * 128
                    if kind == "empty":
                        nc.vector.memset(ps_s[:, col0:col0 + 128], NEG_FILL)
                        continue
                    b0, b1 = bl
                    if kind == "single":
                        nc.tensor.matmul(
                            ps_s[0:64, col0:col0 + 128],
                            kT[hp][:, b0 * 64:(b0 + 1) * 64], qd,
                            start=True, stop=True,
                        )
                        nc.vector.memset(ps_s[64:128, col0:col0 + 128], NEG_FILL)
                    elif kind == "glob":
                        nc.tensor.matmul(
                            ps_s[:, col0:col0 + 128], kg[hp][:], qd,
                            start=True, stop=True,
                        )
                    elif b1 == b0 + 1:
                        nc.tensor.matmul(
                            ps_s[:, col0:col0 + 128],
                            kT[hp][:, b0 * 64:(b0 + 2) * 64], qd,
                            start=True, stop=True,
                        )
                    else:
                        # non-adjacent pair: stationary APs allow only one
                        # free dim, so split into the two output quadrants
                        for sp, blk in enumerate(bl):
                            nc.tensor.matmul(
                                ps_s[sp * 64:sp * 64 + 64, col0:col0 + 128],
                                kT[hp][:, blk * 64:(blk + 1) * 64], qd,
                                start=True, stop=True,
                            )
                p_sb = p_pool.tile([128, 512], BF16, name="p", tag="p")
                nc.scalar.activation(out=p_sb[:], in_=ps_s[:], func=AF.Exp)
                return p_sb

            def attn_stage2(i, hp, p_sb, vp):
                chunks = _row_slots(i)
                pT = [p_sb[:, c * 128:(c + 1) * 128] for c in range(4)]
                mms = [(ci, kind, bl) for ci, (kind, bl) in enumerate(chunks)
                       if kind != "empty"]
                ps_e = psO.tile([128, 129], FP32, name="psO", tag="psO")
                for n_s, (ci, kind, bl) in enumerate(mms):
                    fl = (n_s == 0, n_s == len(mms) - 1)
                    if kind == "glob":
                        rhs = v_glob[:, hp * 129:(hp + 1) * 129]
                    elif kind == "nat":
                        rhs = vcols(v_all, bl[0] // 2, hp)
                    else:
                        rhs = vcols(vp, pair_slot[(i, ci)], hp)
                    nc.tensor.matmul(ps_e[:], pT[ci][:], rhs,
                                     start=fl[0], stop=fl[1])
                pinv = stats.tile([128, 1], FP32, name="pinv", tag="pinv")
                nc.vector.reciprocal(out=pinv[:], in_=ps_e[:, 128:129])
                for h2 in range(2):
                    pr = slice(h2 * 64, h2 * 64 + 64)
                    dst = o_pair[hp][pr, i * 64:(i + 1) * 64]
                    nc.vector.tensor_scalar_mul(
                        out=dst, in0=ps_e[pr, pr], scalar1=pinv[pr],
                    )

            # ---------------- oT transpose + partial outproj + early RS --------
            def emit_oT(blk):
                for hp in range(2):
                    pst = psT.tile([128, 512], BF16, name="psT", tag="psT")
                    for h2 in range(2):
                        pr = slice(h2 * 64, h2 * 64 + 64)
                        nc.tensor.transpose(
                            pst[pr, 0:64],
                            o_pair[hp][pr, blk * 64:(blk + 1) * 64],
                            ident[pr, pr],
                            tile_position=(h2 * 64, h2 * 64),
                        )
                    cp(oT[hp][:, blk * 64:(blk + 1) * 64], pst[:, 0:64])

            def emit_outproj(t):
                ps = psA.tile([128, 512], FP32, name="psA", tag="psA")
                for c in range(2):
                    nc.tensor.matmul(
                        ps[:], oT[c][:, t * 128:(t + 1) * 128],
                        wo_all[:, c * D:(c + 1) * D],
                        start=(c == 0), stop=(c == 1),
                    )
                at = ostream.tile([128, D], BF16, name="at", tag="at")
                if t % 2 == 0:
                    cp(at[:], ps[:])
                else:
                    nc.scalar.copy(out=at[:], in_=ps[:])
                # chunk-major over rs_nchunks: chunk c = rows
                # [c*(S//n), +S//n) = [half0 tokens | half1 tokens] of chunk c
                tpc = 16 // rs_nchunks          # 128-token t-chunks per rs chunk
                if t < 16:
                    row = (t // tpc) * (S // rs_nchunks) + (t % tpc) * 128
                else:
                    row = (((t - 16) // tpc) * (S // rs_nchunks)
                           + TH // rs_nchunks + ((t - 16) % tpc) * 128)
                nc.sync.dma_start(out=rs_in[row:row + 128, :], in_=at[:])

            def emit_rs(c):
                rin = S // rs_nchunks           # input rows per rs chunk
                rout = TH // rs_nchunks
                if no_coll:
                    nc.sync.dma_start(
                        out=rs_out[c * rout:(c + 1) * rout, :],
                        in_=rs_in[c * rin:c * rin + rout, :])
                    return
                nc.gpsimd.collective_compute(
                    "ReduceScatter", ALU.add,
                    replica_groups=[[0, 1], [2, 3], [4, 5], [6, 7]],
                    ins=[rs_in[c * rin:(c + 1) * rin, :]],
                    outs=[rs_out[c * rout:(c + 1) * rout, :]],
                )

            # two-stage pipeline: stage1 (scores+exp) of unit u+1 is emitted
            # before stage2 (AV) of unit u, so Act exp overlaps PE
            prev = [None]

            def flush_prev():
                if prev[0] is not None:
                    (pi, php, pvp), pp = prev[0]
                    attn_stage2(pi, php, pp, pvp)
                    prev[0] = None

            # ---------------- residual + LN2 + FFN per 512 own tokens ----------
            # split so the vector-bound LN2 j-loop can be emitted BEFORE the
            # group's oproj matmuls: the PE chews oproj while DVE does LN2,
            # instead of idling at each ffn chunk start.
            def ffn_ln_vec(ct):
                """Vector/scalar-only LN2 prologue (no PE) so it can be
                emitted before the group's oproj matmuls."""
                r4 = ffn_pool.tile([128, 4, D], BF16, name="r4", tag="r4")
                t2s = []
                for j in range(4):      # 128-token groups
                    a2 = oin_pool.tile([128, D], BF16, name="a2", tag="a2")
                    nc.sync.dma_start(
                        out=a2[:],
                        in_=rs_out.rearrange("(a p) d -> p a d", p=128)[
                            :, ct * 4 + j, :])
                    xo2 = xo_pool.tile([128, D], BF16, name="xo2", tag="xo4")
                    nc.sync.dma_start(
                        out=xo2[:],
                        in_=x_own.rearrange("(a p) d -> p a d", p=128)[
                            :, ct * 4 + j, :])
                    nc.vector.tensor_tensor(out=r4[:, j, :],
                                            in0=a2[:], in1=xo2[:], op=ALU.add)
                    st = stats.tile([128, 6], FP32, name="bst", tag="bst")
                    nc.vector.bn_stats(out=st[:], in_=r4[:, j, :])
                    mv = stats.tile([128, 2], FP32, name="mv", tag="mv")
                    nc.vector.bn_aggr(out=mv[:], in_=st[:])
                    std = stats.tile([128, 1], FP32, name="std", tag="std")
                    nc.scalar.activation(out=std[:], in_=mv[:, 1:2], func=AF.Sqrt,
                                         bias=eps_t[:], scale=1.0)
                    inv = stats.tile([128, 1], FP32, name="inv", tag="inv")
                    nc.vector.reciprocal(out=inv[:], in_=std[:])
                    t2 = tnat_pool.tile([128, D], BF16, name="t2", tag="tn")
                    nc.vector.tensor_scalar(out=t2[:], in0=r4[:, j, :],
                                            scalar1=mv[:, 0:1], scalar2=inv[:],
                                            op0=ALU.subtract, op1=ALU.mult)
                    t2s.append(t2)
                return r4, t2s

            def ffn_ln_tr(t2s):
                t2T_all = ffn_pool.tile([128, 4, 512], wdt, name="t2Tall",
                                        tag="t2Tall")
                for j in range(4):
                    pst = psT.tile([128, 512], BF16, name="psT", tag="psT")
                    for c in range(4):
                        nc.tensor.transpose(pst[:, c * 128:(c + 1) * 128],
                                            t2s[j][:, c * 128:(c + 1) * 128],
                                            ident[:])
                    dst = t2T_all[:, :, j * 128:(j + 1) * 128]
                    if j % 2 == 0:
                        cp(dst, pst[:].rearrange("p (c s) -> p c s", c=4))
                    else:
                        nc.scalar.copy(
                            out=dst, in_=pst[:].rearrange("p (c s) -> p c s", c=4))
                return t2T_all

            def ffn_chunk_ln(ct):
                r4, t2s = ffn_ln_vec(ct)
                return r4, ffn_ln_tr(t2s)

            def ffn_chunk_mm(ct, r4, t2T_all):
                h_all = h_pool.tile([128, 16, 512], wdt, name="hall", tag="hall")
                for mt in range(16):
                    ps = psA.tile([128, 512], FP32, name="psA", tag="psA")
                    if ffn_fp8:
                        for d2 in range(2):
                            nc.tensor.matmul(
                                ps[:],
                                w1v[:, 2 * d2:2 * d2 + 2, mt * 128:(mt + 1) * 128],
                                t2T_all[:, 2 * d2:2 * d2 + 2, :],
                                start=(d2 == 0), stop=(d2 == 1), perf_mode=DR,
                            )
                    else:
                        for d in range(4):
                            nc.tensor.matmul(
                                ps[:], w1v[:, d, mt * 128:(mt + 1) * 128],
                                t2T_all[:, d, :],
                                start=(d == 0), stop=(d == 3),
                            )
                    nc.scalar.activation(out=h_all[:, mt, :], in_=ps[:],
                                         func=AF.Gelu_apprx_tanh,
                                         bias=b1_sb[:, mt:mt + 1],
                                         scale=(1.0 / W8 if ffn_fp8 else 1.0))
                for j in range(4):
                    ps2 = psA.tile([128, 512], FP32, name="psA", tag="psA")
                    if ffn_fp8:
                        for m2 in range(8):
                            nc.tensor.matmul(
                                ps2[:],
                                h_all[:, 2 * m2:2 * m2 + 2, j * 128:(j + 1) * 128],
                                w2v[:, 2 * m2:2 * m2 + 2, :],
                                start=(m2 == 0), stop=(m2 == 7), perf_mode=DR,
                            )
                    else:
                        for mt in range(16):
                            nc.tensor.matmul(
                                ps2[:], h_all[:, mt, j * 128:(j + 1) * 128],
                                w2v[:, mt, :],
                                start=(mt == 0), stop=(mt == 15),
                            )
                    y1 = ostream.tile([128, D], FP32, name="y1", tag="y1")
                    if ffn_fp8:
                        nc.vector.scalar_tensor_tensor(
                            out=y1[:], in0=ps2[:], scalar=1.0 / W8,
                            in1=b2_rep[:], op0=ALU.mult, op1=ALU.add)
                    else:
                        nc.vector.tensor_tensor(out=y1[:], in0=ps2[:],
                                                in1=b2_rep[:], op=ALU.add)
                    nc.vector.tensor_tensor(out=y1[:], in0=y1[:],
                                            in1=r4[:, j, :], op=ALU.add)
                    nc.sync.dma_start(
                        out=out_d.rearrange("(a p) d -> p a d", p=128)[:, ct * 4 + j, :],
                        in_=y1[:])

            # ---------------- main schedule ----------------
            def emit_attn_group(c, half, ffn_ct):
                blk0 = half * 32 + 8 * c
                qd = build_qd_grp(blk0)
                vp = build_gathers(blk0 // 8)
                with nc.named_scope(f"attn{c}{half}"):
                    for i in range(blk0, blk0 + 8):
                        for hp in range(2):
                            if i in (0, NB - 1):
                                flush_prev()
                                emit_global(i, hp, qd_slice(qd, blk0, i, hp))
                            else:
                                cur = ((i, hp, vp),
                                       attn_stage1(i, hp, qd_slice(qd, blk0, i, hp)))
                                flush_prev()
                                prev[0] = cur
                    flush_prev()
                ffn_st = None
                if ffn_ct is not None:
                    with nc.named_scope(f"ffn{ffn_ct}a"):
                        ffn_st = ffn_ln_vec(ffn_ct)
                with nc.named_scope(f"oproj{c}{half}"):
                    for i in range(blk0, blk0 + 8):
                        emit_oT(i)
                    t0 = 16 * half + 4 * c
                    for t in range(t0, t0 + 4):
                        emit_outproj(t)
                if ffn_ct is not None:
                    r4, t2s = ffn_st
                    with nc.named_scope(f"ffn{ffn_ct}b"):
                        ffn_chunk_mm(ffn_ct, r4, ffn_ln_tr(t2s))

            rs_issued = [0]
            next_ffn = [0]
            rs_hg = {}
            hg = [0]

            def emit_ffn_next():
                with nc.named_scope(f"ffn{next_ffn[0]}"):
                    ffn_chunk_mm(next_ffn[0], *ffn_chunk_ln(next_ffn[0]))
                next_ffn[0] += 1

            def ffn_ready(hg_now):
                # ffn k once rs k issued >= 2 half-groups (~50us) ago:
                # the CC takes ~20us queue + ~18us to run, and an ffn
                # emitted too early stalls every later vector op behind
                # its rs_out wait (engine streams are in-order)
                return (next_ffn[0] < 4 and next_ffn[0] < rs_issued[0]
                        and hg_now > rs_hg[next_ffn[0]] + (ffn_slack - 1))

            for c in range(4):
                for half in range(2):
                    ct = next_ffn[0] if ffn_ready(hg[0] + 1) else None
                    emit_attn_group(c, half, ct)
                    if ct is not None:
                        next_ffn[0] += 1
                    hg[0] += 1
                    while ffn_ready(hg[0]):
                        emit_ffn_next()
                for jr in range(rs_nchunks // 4):
                    emit_rs(c * (rs_nchunks // 4) + jr)
                rs_issued[0] = c + 1
                rs_hg[c] = hg[0]
            while next_ffn[0] < 4:
                emit_ffn_next()

    nc.compile()
    return nc


_NC_CACHE = None


def _get_nc():
    global _NC_CACHE
    if _NC_CACHE is None:
        _NC_CACHE = build_kernel()
    return _NC_CACHE


def prepare_in_maps(inputs, ffn_fp8=False):
    inp = {k: np.asarray(v, dtype=np.float32) for k, v in inputs.items()}
    x = inp["inputs"]                    # [B, S, D]
    s1, bb1 = inp["ln1_scale"], inp["ln1_bias"]
    s2, bb2 = inp["ln2_scale"], inp["ln2_bias"]
    wq, wk, wv = inp["wq"], inp["wk"], inp["wv"]          # [D, H, DH]
    wo = inp["wo"]                                        # [H, DH, D]
    w1, b1, w2, b2 = inp["w1"], inp["b1"], inp["w2"], inp["b2"]

    scale = 1.0 / np.sqrt(np.float32(DH))
    bf = ml_dtypes.bfloat16
    f8 = ml_dtypes.float8_e4m3

    def to8(a):
        return np.clip(a * W8, -240.0, 240.0).astype(f8)

    in_maps = []
    for p in range(8):
        b = p // 2
        j = p % 2
        hs = slice(4 * j, 4 * j + 4)
        wq_l = wq[:, hs, :].reshape(D, F)
        wk_l = wk[:, hs, :].reshape(D, F)
        wv_l = wv[:, hs, :].reshape(D, F)
        wq_eff = (s1[:, None] * wq_l * scale).astype(bf)
        wk_eff = (s1[:, None] * wk_l).astype(bf)
        wv_eff = (s1[:, None] * wv_l).astype(bf)
        qb = (bb1 @ wq_l * scale).astype(np.float32)
        kb = (bb1 @ wk_l).astype(np.float32)
        wo_l = wo[hs].reshape(F, D).astype(bf)
        w1_sc = s2[:, None] * w1
        b1_eff = (b1 + bb2 @ w1).astype(np.float32)
        # v bias folded into the residual input: o_final = (sum p v)/r + vb,
        # and vb @ wo (all heads) is a constant per-token row added here
        vb_full = bb1 @ wv.reshape(D, H * DH)           # [H*DH]
        vbwo = vb_full @ wo.reshape(H * DH, D)          # [D]
        in_maps.append({
            "x_full": np.ascontiguousarray(x[b]).astype(bf),
            "x_own": (np.ascontiguousarray(x[b, j * TH:(j + 1) * TH]) + vbwo).astype(bf),
            "wq": wq_eff, "wk": wk_eff, "wv": wv_eff, "wo": wo_l,
            "w1": to8(w1_sc) if ffn_fp8 else w1_sc.astype(bf),
            "w2": to8(w2) if ffn_fp8 else w2.astype(bf),
            "qb": qb, "kb": kb,
            "b1": b1_eff, "b2": b2.astype(np.float32),
        })
    return in_maps


def kernel(**inputs):
    in_maps = prepare_in_maps(inputs)
    nc = _get_nc()
    try:
        res = bass_utils.run_bass_kernel_spmd(nc, in_maps, core_ids=list(range(8)))
    except Exception:
        # transient NRT_EXEC_UNIT_UNRECOVERABLE wedges recover on retry
        import time as _time
        _time.sleep(2.0)
        res = bass_utils.run_bass_kernel_spmd(nc, in_maps, core_ids=list(range(8)))
    out = np.zeros((B, S, D), np.float32)
    for p in range(8):
        b, j = p // 2, p % 2
        out[b, j * TH:(j + 1) * TH] = res.results[p]["out"]
    return out


if __name__ == "__main__":
    nc = _get_nc()
    print("built ok")


# revision 73
# speedup vs baseline: 2.5986x; 2.5986x over previous
"""BigBird block kernel for 8 Trainium2 NeuronCores (v3).

Sharding: core p in 0..7 -> batch b = p//2, head-half j = p%2 (heads 4j..4j+3).
Each core: LN1 over its full batch-row, q/k/v for its 4 heads over the full
sequence, block-sparse attention for all 64 query blocks (its heads), then a
partial output projection.  The projection partials are pairwise
ReduceScattered in 4 token chunks, each issued as soon as its attention
blocks finish so the collectives overlap the remaining attention; FFN chunks
are interleaved into the schedule one RS chunk behind.  Residual + LN2 + FFN
run on the core's own 2048-token half; the host reassembles 8 halves.

v3 changes vs v2:
- Scores are computed TRANSPOSED (sT[j, q] = kT_block.T @ qd): the exp
  output pT feeds the AV matmuls directly, eliminating all per-unit PE
  transposes and the pt_sb copies.
- Softmax row sums come from a ones column baked into the v tiles (group
  width 258 = 2 heads x (128 feat + 1)); normalization divides the AV
  output rows as before.
- The FFN runs in fp8(e4m3) DoubleRow perf mode (weights pre-scaled by 64
  host-side, rescaled in the activation), halving PE time there.
- Phase-1 engine rebalance: transpose writebacks go to GpSimd, v copies
  alternate Vector/GpSimd.
All other matmuls bf16 with fp32 PSUM accumulation.
"""

import numpy as np
import ml_dtypes

import concourse.bass as bass
import concourse.bacc as bacc
import concourse.tile as tile
from concourse import mybir
from concourse import masks
from concourse import bass_utils

# ---- problem constants (hardcoded; must match the reference) ----
B, S, D = 4, 4096, 512
H, DH = 8, 64
M = 2048
BS = 64
NB = S // BS            # 64 blocks
R = 3
K = 8
SEED = 0
LN_EPS = 1e-6

HL = 4                  # local heads per core
F = HL * DH             # 256 local qkv features
TH = S // 2             # 2048 tokens per core half
NEG_FILL = -30.0        # exp(-30) ~ 1e-13: masked-slot fill before exp
VG = 258                # v group width: 2 hp x (128 features + ones col)
W8 = 64.0               # host-side fp8 weight pre-scale for the FFN

FP32 = mybir.dt.float32
BF16 = mybir.dt.bfloat16
F8 = mybir.dt.float8e4
AF = mybir.ActivationFunctionType
ALU = mybir.AluOpType
AX = mybir.AxisListType
DR = mybir.MatmulPerfMode.DoubleRow


def _block_indices():
    """Static BigBird connectivity (identical to the reference)."""
    rng = np.random.RandomState(SEED)
    idx = np.zeros((NB - 2, K), np.int32)
    msk = np.zeros((NB - 2, K), bool)
    for row, i in enumerate(range(1, NB - 1)):
        fixed = sorted({0, NB - 1, i - 1, i, i + 1})
        allowed = [b for b in range(NB) if b not in fixed]
        rand = rng.choice(allowed, size=R, replace=False).tolist()
        blocks = fixed + rand
        idx[row, :len(blocks)] = blocks
        msk[row, :len(blocks)] = True
    return idx, msk


IDX, KMASK = _block_indices()


def _group_pairs(gsz=4):
    """Per gsz-block half-group: the (row, chunk) slots whose key/value
    blocks are non-adjacent ('pair') or padded ('single'), each gathered
    into a [128, VG] v tile (block lo -> rows 0-63, hi -> rows 64-127;
    singles duplicate their block, the dup side sees only exp(NEG_FILL))
    and, per hp, a [128, 128] k tile for one contiguous score matmul
    (singles keep the quadrant score path + NEG_FILL memset).
    Returns {grp: [(i, ci, b0, b1, kind), ...]} and {(i, ci): slot}."""
    groups, slots = {}, {}
    for i in range(1, NB - 1):
        for ci, (kind, bl) in enumerate(_row_slots(i)):
            if kind == "pair" or kind == "single":
                b0, b1 = bl
                if b1 is None:
                    b1 = b0
                g = i // gsz
                lst = groups.setdefault(g, [])
                slots[(i, ci)] = len(lst)
                lst.append((i, ci, b0, b1, kind))
    return groups, slots


def _row_slots(i):
    """Slot layout for interior block i: list of (kind, blocks) chunk pairs.

    Each chunk covers 2 of the 8 score-column slots.  kind: 'nat' = natural
    v pair (even,even+1), 'glob' = (0,63) via v_glob, 'pair' = 2 unrelated
    singles, 'single' = one block (+pad).  Returns a list of 4 entries, each
    (kind, [b0, b1]) with b1 None for a padded single.
    """
    row = i - 1
    bs = sorted(int(IDX[row, s]) for s in range(K) if KMASK[row, s])
    rest = [b for b in bs if b not in (0, NB - 1)]
    nat, singles = [], []
    j = 0
    while j < len(rest):
        if j + 1 < len(rest) and rest[j] % 2 == 0 and rest[j + 1] == rest[j] + 1:
            nat.append((rest[j], rest[j + 1]))
            j += 2
        else:
            singles.append(rest[j])
            j += 1
    chunks = [("glob", [0, NB - 1])]
    for p in nat:
        chunks.append(("nat", list(p)))
    j = 0
    while j < len(singles):
        if j + 1 < len(singles):
            chunks.append(("pair", [singles[j], singles[j + 1]]))
            j += 2
        else:
            chunks.append(("single", [singles[j], None]))
            j += 1
    assert len(chunks) <= 4
    while len(chunks) < 4:
        chunks.append(("empty", [None, None]))
    return chunks


def build_kernel(rs_nchunks=4, no_coll=False, ffn_fp8=False, ffn_slack=2,
                 psa_bufs=4, pp_bufs=3):
    nc = bacc.Bacc(
        "TRN2",
        target_bir_lowering=False,
        debug=False,
        num_devices=8,
    )

    wdt = F8 if ffn_fp8 else BF16

    # ---- DRAM I/O ----
    x_full = nc.dram_tensor("x_full", [S, D], BF16, kind="ExternalInput").ap()
    x_own = nc.dram_tensor("x_own", [TH, D], BF16, kind="ExternalInput").ap()
    wq_d = nc.dram_tensor("wq", [D, F], BF16, kind="ExternalInput").ap()
    wk_d = nc.dram_tensor("wk", [D, F], BF16, kind="ExternalInput").ap()
    wv_d = nc.dram_tensor("wv", [D, F], BF16, kind="ExternalInput").ap()
    wo_d = nc.dram_tensor("wo", [F, D], BF16, kind="ExternalInput").ap()
    w1_d = nc.dram_tensor("w1", [D, M], wdt, kind="ExternalInput").ap()
    w2_d = nc.dram_tensor("w2", [M, D], wdt, kind="ExternalInput").ap()
    qb_d = nc.dram_tensor("qb", [F], FP32, kind="ExternalInput").ap()
    kb_d = nc.dram_tensor("kb", [F], FP32, kind="ExternalInput").ap()
    b1_d = nc.dram_tensor("b1", [M], FP32, kind="ExternalInput").ap()
    b2_d = nc.dram_tensor("b2", [D], FP32, kind="ExternalInput").ap()
    out_d = nc.dram_tensor("out", [TH, D], FP32, kind="ExternalOutput").ap()

    rs_in = nc.dram_tensor("rs_in", [S, D], BF16).ap()
    rs_out = nc.dram_tensor("rs_out", [TH, D], BF16).ap()

    with tile.TileContext(nc) as tc:
        with (
            tc.tile_pool(name="const", bufs=1) as const,
            tc.tile_pool(name="persist", bufs=1) as persist,
            tc.tile_pool(name="xin", bufs=3) as xin,
            tc.tile_pool(name="xo_pool", bufs=2) as xo_pool,
            tc.tile_pool(name="stats", bufs=6) as stats,
            tc.tile_pool(name="tnat", bufs=5) as tnat_pool,
            tc.tile_pool(name="vpair", bufs=2) as vpair_pool,
            tc.tile_pool(name="psA", bufs=psa_bufs, space="PSUM") as psA,
            tc.tile_pool(name="psT", bufs=2, space="PSUM") as psT,
            tc.tile_pool(name="psO", bufs=2, space="PSUM") as psO,
            tc.tile_pool(name="p_pool", bufs=pp_bufs) as p_pool,
            tc.tile_pool(name="ostream", bufs=2) as ostream,
            tc.tile_pool(name="oin_pool", bufs=2) as oin_pool,
            tc.tile_pool(name="qd_pool", bufs=2) as qd_pool,
            tc.tile_pool(name="ffn", bufs=2) as ffn_pool,
            tc.tile_pool(name="hpool", bufs=1) as h_pool,
        ):
            # ---------------- constants ----------------
            ident = const.tile([128, 128], BF16, name="ident", tag="ident")
            masks.make_identity(nc, ident[:])

            eps_t = const.tile([128, 1], FP32, name="eps", tag="eps")
            nc.vector.memset(eps_t[:], LN_EPS)

            def load_cols(dram_ap, n, tag):
                t = const.tile([128, n // 128], FP32, name=tag, tag=tag)
                nc.sync.dma_start(out=t[:], in_=dram_ap.rearrange("(t p) -> p t", p=128))
                return t

            qb_sb = load_cols(qb_d, F, "qb")
            kb_sb = load_cols(kb_d, F, "kb")
            b1_sb = load_cols(b1_d, M, "b1")

            b2_rep = const.tile([128, D], FP32, name="b2rep", tag="b2rep")
            nc.sync.dma_start(
                out=b2_rep[:],
                in_=bass.AP(tensor=b2_d.tensor, offset=b2_d.offset,
                            ap=[[0, 128]] + b2_d.ap),
            )

            # weights: one SBUF tile per matrix, d-tile-major columns, 1 DMA each
            def load_w(dram_ap, rows, cols, tag, dt=BF16):
                nt = rows // 128
                t = const.tile([128, nt * cols], dt, name=tag, tag=tag)
                nc.sync.dma_start(
                    out=t[:].rearrange("p (t f) -> p t f", t=nt),
                    in_=dram_ap.rearrange("(t p) f -> p t f", p=128))
                return t

            # early weights only: q/k/v are needed from the first qk chunk.
            # wo/w1/w2 (5 MB) are DMA'd after phase 1 so they don't queue
            # ahead of the x tiles on the HBM path.
            wq_all = load_w(wq_d, D, F, "wqall")     # [:, d*F + f]
            wk_all = load_w(wk_d, D, F, "wkall")
            wv_all = load_w(wv_d, D, F, "wvall")
            # persistent activations
            big_all = persist.tile([128, 4 * S], BF16, name="bigall", tag="bigall")
            tT = [big_all[:, d * S:(d + 1) * S] for d in range(4)]
            qT = [persist.tile([128, S], BF16, name=f"qT{c}", tag=f"qT{c}")
                  for c in range(2)]
            kT = [persist.tile([128, S], BF16, name=f"kT{c}", tag=f"kT{c}")
                  for c in range(2)]
            v_all = persist.tile([128, 32 * VG], BF16, name="vall", tag="vall")
            group_pairs, pair_slot = _group_pairs(8)
            npair_max = max(len(v) for v in group_pairs.values())
            # o_pair reuses big_all cols [0, 2S); oT (local heads) in [2S, 4S)
            o_pair = [big_all[:, c * S:(c + 1) * S] for c in range(2)]
            oT = [big_all[:, (2 + c) * S:(3 + c) * S] for c in range(2)]
            v_glob = persist.tile([128, VG], BF16, name="vglob", tag="vglob")
            kg = [persist.tile([128, 128], BF16, name=f"kg{c}", tag=f"kg{c}")
                  for c in range(2)]

            vv = v_all[:].rearrange("p (g h c) -> p g h c", g=32, h=2)
            # ones columns (rowsum accumulators) in v_all; v_sh copies them
            for hp in range(2):
                nc.vector.memset(vv[:, :, hp, 128:129], 1.0)

            def vcols(t, g, hp):
                return t[:, g * VG + hp * 129:g * VG + hp * 129 + 129]

            def cp(out, in_):
                nc.vector.tensor_copy(out=out, in_=in_)

            # ---------------- phase 2 helpers: q/k/v projections ----------------
            def emit_qk_chunk(tck):
                for w_sb, bias_sb, dstT in ((wq_all, qb_sb, qT), (wk_all, kb_sb, kT)):
                    for ft in range(2):
                        ps = psA.tile([128, 512], FP32, name="psA", tag="psA")
                        for d in range(4):
                            nc.tensor.matmul(
                                ps[:], w_sb[:, d * F + ft * 128:d * F + (ft + 1) * 128],
                                tT[d][:, tck * 512:(tck + 1) * 512],
                                start=(d == 0), stop=(d == 3),
                            )
                        if ft == 0:
                            nc.scalar.activation(
                                out=dstT[ft][:, tck * 512:(tck + 1) * 512],
                                in_=ps[:], func=AF.Identity,
                                bias=bias_sb[:, ft:ft + 1], scale=1.0,
                            )
                        else:
                            nc.vector.tensor_scalar_add(
                                out=dstT[ft][:, tck * 512:(tck + 1) * 512],
                                in0=ps[:], scalar1=bias_sb[:, ft:ft + 1],
                            )

            # v bias is folded into x_own host-side (softmax weights sum to 1,
            # so o = (sum p v)/r + vb exactly; vb@wo is a constant per token)
            def emit_v_tile(tt):
                ps = psA.tile([128, 512], FP32, name="psA", tag="psA")
                for d in range(4):
                    nc.tensor.matmul(
                        ps[:, 0:F], tT[d][:, tt * 128:(tt + 1) * 128],
                        wv_all[:, d * F:(d + 1) * F],
                        start=(d == 0), stop=(d == 3),
                    )
                dst = vv[:, tt, :, 0:128]
                src = ps[:, 0:F].rearrange("p (h c) -> p h c", h=2)
                if tt % 2 == 0:
                    cp(dst, src)
                else:
                    nc.scalar.copy(out=dst, in_=src)

            # ---------------- phase 1: LN1 + transpose ----------------
            def ln1_tile(xt, tt, j):
                st = stats.tile([128, 6], FP32, name="bst", tag="bst")
                nc.vector.bn_stats(out=st[:], in_=xt)
                mv = stats.tile([128, 2], FP32, name="mv", tag="mv")
                nc.vector.bn_aggr(out=mv[:], in_=st[:])
                std = stats.tile([128, 1], FP32, name="std", tag="std")
                nc.scalar.activation(out=std[:], in_=mv[:, 1:2], func=AF.Sqrt,
                                     bias=eps_t[:], scale=1.0)
                inv = stats.tile([128, 1], FP32, name="inv", tag="inv")
                nc.vector.reciprocal(out=inv[:], in_=std[:])
                tn = tnat_pool.tile([128, D], BF16, name="tn", tag="tn")
                if j == 0:
                    nmi = stats.tile([128, 1], FP32, name="nmi", tag="nmi")
                    nc.vector.tensor_scalar(out=nmi[:], in0=mv[:, 0:1],
                                            scalar1=inv[:], scalar2=-1.0,
                                            op0=ALU.mult, op1=ALU.mult)
                    nc.scalar.activation(out=tn[:], in_=xt, func=AF.Identity,
                                         bias=nmi[:], scale=inv[:])
                else:
                    nc.vector.tensor_scalar(out=tn[:], in0=xt,
                                            scalar1=mv[:, 0:1], scalar2=inv[:],
                                            op0=ALU.subtract, op1=ALU.mult)
                pst = psT.tile([128, 512], BF16, name="psT", tag="psT")
                for c in range(4):
                    nc.tensor.transpose(pst[:, c * 128:(c + 1) * 128],
                                        tn[:, c * 128:(c + 1) * 128], ident[:])
                dst = big_all[:].rearrange("p (c s) -> p c s", c=4)[
                    :, :, tt * 128:(tt + 1) * 128]
                if j == 0:
                    cp(dst, pst[:].rearrange("p (c s) -> p c s", c=4))
                else:
                    nc.scalar.copy(
                        out=dst, in_=pst[:].rearrange("p (c s) -> p c s", c=4))

            with nc.named_scope("ph1"):
                for tq in range(16):
                    xt4 = xin.tile([128, 2, D], BF16, name="xt4", tag="xt4")
                    nc.sync.dma_start(
                        out=xt4[:],
                        in_=x_full.rearrange("(a p) d -> p a d", p=128)[:, tq * 2:(tq + 1) * 2, :])
                    for j in range(2):
                        ln1_tile(xt4[:, j, :], tq * 2 + j, j)
                    if tq % 2 == 1:
                        k4 = tq // 2
                        emit_qk_chunk(k4)
                        for vt in range(4 * k4, 4 * k4 + 4):
                            emit_v_tile(vt)

                # v_glob: block 0 (top of group 0) / block 63 (bottom of grp 31)
                cp(v_glob[0:64, :], v_all[0:64, 0:VG])
                cp(v_glob[64:128, :], v_all[64:128, 31 * VG:32 * VG])
                # kg: global key pair (block 0 | block 63) gathered adjacent so
                # the glob score chunk is one contiguous stationary matmul
                for hp in range(2):
                    cp(kg[hp][:, 0:64], kT[hp][:, 0:64])
                    cp(kg[hp][:, 64:128], kT[hp][:, 63 * 64:64 * 64])

            # late weights (first used >=150us in): wo at the first outproj,
            # w1/w2/b1/b2 at the first ffn chunk
            wo_all = load_w(wo_d, F, D, "woall")     # [:, c*D + e], c = f-chunk
            w1_all = load_w(w1_d, D, M, "w1all", wdt)   # [:, d*M + m]
            w2_all = load_w(w2_d, M, D, "w2all", wdt)   # [:, m*D + e]
            w1v = w1_all[:].rearrange("p (d m) -> p d m", d=4)
            w2v = w2_all[:].rearrange("p (m e) -> p m e", m=16)

            # block-diagonal q tiles, built per 8-block group on GpSimd.
            # Layout: [128, 2 hp x 8 blocks x 128].  Used as the MOVING
            # operand of the transposed score matmuls.
            qd_gen = [0]

            def build_qd_grp(blk0):
                eng = nc.gpsimd
                qd = qd_pool.tile([128, 2 * 8 * 128], BF16, name="qd", tag="qd")
                # zeros persist across pool rotation; same-engine program
                # order makes gen>=2 reads safe without re-zeroing
                if qd_gen[0] < 2:
                    eng.memset(qd[:], 0.0)
                qd_gen[0] += 1
                # block-major, hp inner: matches the (block, hp) consumption
                # order, so the first units don't wait on the whole build
                for ir in range(8):
                    i = blk0 + ir
                    for hp in range(2):
                        for h2 in range(2):
                            pr = slice(h2 * 64, h2 * 64 + 64)
                            c0 = (hp * 8 + ir) * 128 + h2 * 64
                            eng.tensor_copy(
                                out=qd[pr, c0:c0 + 64],
                                in_=qT[hp][pr, i * 64:(i + 1) * 64],
                            )
                return qd

            def qd_slice(qd, blk0, i, hp):
                c0 = (hp * 8 + (i - blk0)) * 128
                return qd[:, c0:c0 + 128]

            # per-group gathered v tiles for 'pair'/'single' chunks: block lo
            # -> rows 0-63, hi -> rows 64-127 (both hp slices + ones cols),
            # so every AV matmul is one full-128 chain link.
            def build_gathers(grp):
                pairs = group_pairs.get(grp, [])
                vp = vpair_pool.tile([128, npair_max * VG], BF16,
                                     name="vp", tag="vp")
                for slot, (i, ci, b0, b1, kind) in enumerate(pairs):
                    for sp, blk in enumerate((b0, b1)):
                        nc.sync.dma_start(
                            out=vp[sp * 64:sp * 64 + 64,
                                   slot * VG:(slot + 1) * VG],
                            in_=v_all[(blk % 2) * 64:(blk % 2) * 64 + 64,
                                      (blk // 2) * VG:(blk // 2 + 1) * VG])
                return vp

            # ---------------- global block unit (full attention) ----------------
            def emit_global(g, hp, qd):
                ps_og = psO.tile([128, 129], FP32, name="psO", tag="psO")
                for kc in range(8):
                    ps_s = psA.tile([128, 512], FP32, name="psA", tag="psA")
                    for c in range(4):
                        j0 = (kc * 4 + c) * 128
                        nc.tensor.matmul(
                            ps_s[:, c * 128:(c + 1) * 128],
                            kT[hp][:, j0:j0 + 128], qd,
                            start=True, stop=True,
                        )
                    p_sb = p_pool.tile([128, 512], BF16, name="p_g", tag="p")
                    nc.scalar.activation(out=p_sb[:], in_=ps_s[:], func=AF.Exp)
                    for c in range(4):
                        sc = kc * 4 + c
                        nc.tensor.matmul(
                            ps_og[:], p_sb[:, c * 128:(c + 1) * 128],
                            vcols(v_all, sc, hp),
                            start=(sc == 0), stop=(sc == 31),
                        )
                pinv = stats.tile([128, 1], FP32, name="pinv", tag="pinv")
                nc.vector.reciprocal(out=pinv[:], in_=ps_og[:, 128:129])
                for h2 in range(2):
                    pr = slice(h2 * 64, h2 * 64 + 64)
                    dst = o_pair[hp][pr, g * 64:(g + 1) * 64]
                    nc.vector.tensor_scalar_mul(
                        out=dst, in0=ps_og[pr, pr], scalar1=pinv[pr],
                    )

            # ---------------- interior block-sparse attention ----------------
            def attn_stage1(i, hp, qd):
                """Transposed scores sT[j, q] for both heads in one matmul per
                2-block chunk (stationary kT cols, moving block-diag qd)."""
                chunks = _row_slots(i)
                ps_s = psA.tile([128, 512], FP32, name="psA", tag="psA")
                for ci, (kind, bl) in enumerate(chunks):
                    col0 = ci * 128
                    if kind == "empty":
                        nc.vector.memset(ps_s[:, col0:col0 + 128], NEG_FILL)
                        continue
                    b0, b1 = bl
                    if kind == "single":
                        nc.tensor.matmul(
                            ps_s[0:64, col0:col0 + 128],
                            kT[hp][:, b0 * 64:(b0 + 1) * 64], qd,
                            start=True, stop=True,
                        )
                        nc.vector.memset(ps_s[64:128, col0:col0 + 128], NEG_FILL)
                    elif kind == "glob":
                        nc.tensor.matmul(
                            ps_s[:, col0:col0 + 128], kg[hp][:], qd,
                            start=True, stop=True,
                        )
                    elif b1 == b0 + 1:
                        nc.tensor.matmul(
                            ps_s[:, col0:col0 + 128],
                            kT[hp][:, b0 * 64:(b0 + 2) * 64], qd,
                            start=True, stop=True,
                        )
                    else:
                        # non-adjacent pair: stationary APs allow only one
                        # free dim, so split into the two output quadrants
                        for sp, blk in enumerate(bl):
                            nc.tensor.matmul(
                                ps_s[sp * 64:sp * 64 + 64, col0:col0 + 128],
                                kT[hp][:, blk * 64:(blk + 1) * 64], qd,
                                start=True, stop=True,
                            )
                p_sb = p_pool.tile([128, 512], BF16, name="p", tag="p")
                nc.scalar.activation(out=p_sb[:], in_=ps_s[:], func=AF.Exp)
                return p_sb

            def attn_stage2(i, hp, p_sb, vp):
                chunks = _row_slots(i)
                pT = [p_sb[:, c * 128:(c + 1) * 128] for c in range(4)]
                mms = [(ci, kind, bl) for ci, (kind, bl) in enumerate(chunks)
                       if kind != "empty"]
                ps_e = psO.tile([128, 129], FP32, name="psO", tag="psO")
                for n_s, (ci, kind, bl) in enumerate(mms):
                    fl = (n_s == 0, n_s == len(mms) - 1)
                    if kind == "glob":
                        rhs = v_glob[:, hp * 129:(hp + 1) * 129]
                    elif kind == "nat":
                        rhs = vcols(v_all, bl[0] // 2, hp)
                    else:
                        rhs = vcols(vp, pair_slot[(i, ci)], hp)
                    nc.tensor.matmul(ps_e[:], pT[ci][:], rhs,
                                     start=fl[0], stop=fl[1])
                pinv = stats.tile([128, 1], FP32, name="pinv", tag="pinv")
                nc.vector.reciprocal(out=pinv[:], in_=ps_e[:, 128:129])
                for h2 in range(2):
                    pr = slice(h2 * 64, h2 * 64 + 64)
                    dst = o_pair[hp][pr, i * 64:(i + 1) * 64]
                    nc.vector.tensor_scalar_mul(
                        out=dst, in0=ps_e[pr, pr], scalar1=pinv[pr],
                    )

            # ---------------- oT transpose + partial outproj + early RS --------
            def emit_oT(blk):
                for hp in range(2):
                    pst = psT.tile([128, 512], BF16, name="psT", tag="psT")
                    for h2 in range(2):
                        pr = slice(h2 * 64, h2 * 64 + 64)
                        nc.tensor.transpose(
                            pst[pr, 0:64],
                            o_pair[hp][pr, blk * 64:(blk + 1) * 64],
                            ident[pr, pr],
                            tile_position=(h2 * 64, h2 * 64),
                        )
                    cp(oT[hp][:, blk * 64:(blk + 1) * 64], pst[:, 0:64])

            def emit_outproj4(t0):
                """Four consecutive 128-token outproj tiles; their rs_in rows
                are contiguous, so one batched DMA replaces four (HWDGE
                descriptors are ~0.6us each and serialize)."""
                at4 = ostream.tile([128, 2, D], BF16, name="at4", tag="at4")
                for jt in range(2):
                    t = t0 + jt
                    ps = psA.tile([128, 512], FP32, name="psA", tag="psA")
                    for c in range(2):
                        nc.tensor.matmul(
                            ps[:], oT[c][:, t * 128:(t + 1) * 128],
                            wo_all[:, c * D:(c + 1) * D],
                            start=(c == 0), stop=(c == 1),
                        )
                    if t % 2 == 0:
                        cp(at4[:, jt, :], ps[:])
                    else:
                        nc.scalar.copy(out=at4[:, jt, :], in_=ps[:])
                tpc = 16 // rs_nchunks          # 128-token t-chunks per rs chunk
                if t0 < 16:
                    row = (t0 // tpc) * (S // rs_nchunks) + (t0 % tpc) * 128
                else:
                    row = (((t0 - 16) // tpc) * (S // rs_nchunks)
                           + TH // rs_nchunks + ((t0 - 16) % tpc) * 128)
                nc.sync.dma_start(
                    out=rs_in[row:row + 256, :].rearrange(
                        "(a p) d -> p a d", p=128),
                    in_=at4[:])

            def emit_rs(c):
                rin = S // rs_nchunks           # input rows per rs chunk
                rout = TH // rs_nchunks
                if no_coll:
                    nc.sync.dma_start(
                        out=rs_out[c * rout:(c + 1) * rout, :],
                        in_=rs_in[c * rin:c * rin + rout, :])
                    return
                nc.gpsimd.collective_compute(
                    "ReduceScatter", ALU.add,
                    replica_groups=[[0, 1], [2, 3], [4, 5], [6, 7]],
                    ins=[rs_in[c * rin:(c + 1) * rin, :]],
                    outs=[rs_out[c * rout:(c + 1) * rout, :]],
                )

            # two-stage pipeline: stage1 (scores+exp) of unit u+1 is emitted
            # before stage2 (AV) of unit u, so Act exp overlaps PE
            prev = [None]

            def flush_prev():
                if prev[0] is not None:
                    (pi, php, pvp), pp = prev[0]
                    attn_stage2(pi, php, pp, pvp)
                    prev[0] = None

            # ---------------- residual + LN2 + FFN per 512 own tokens ----------
            # split so the vector-bound LN2 j-loop can be emitted BEFORE the
            # group's oproj matmuls: the PE chews oproj while DVE does LN2,
            # instead of idling at each ffn chunk start.
            def ffn_ln_vec(ct):
                """Vector/scalar-only LN2 prologue (no PE) so it can be
                emitted before the group's oproj matmuls."""
                r4 = ffn_pool.tile([128, 4, D], BF16, name="r4", tag="r4")
                t2s = []
                for j in range(4):      # 128-token groups
                    a2 = oin_pool.tile([128, D], BF16, name="a2", tag="a2")
                    nc.sync.dma_start(
                        out=a2[:],
                        in_=rs_out.rearrange("(a p) d -> p a d", p=128)[
                            :, ct * 4 + j, :])
                    xo2 = xo_pool.tile([128, D], BF16, name="xo2", tag="xo4")
                    nc.sync.dma_start(
                        out=xo2[:],
                        in_=x_own.rearrange("(a p) d -> p a d", p=128)[
                            :, ct * 4 + j, :])
                    nc.vector.tensor_tensor(out=r4[:, j, :],
                                            in0=a2[:], in1=xo2[:], op=ALU.add)
                    st = stats.tile([128, 6], FP32, name="bst", tag="bst")
                    nc.vector.bn_stats(out=st[:], in_=r4[:, j, :])
                    mv = stats.tile([128, 2], FP32, name="mv", tag="mv")
                    nc.vector.bn_aggr(out=mv[:], in_=st[:])
                    std = stats.tile([128, 1], FP32, name="std", tag="std")
                    nc.scalar.activation(out=std[:], in_=mv[:, 1:2], func=AF.Sqrt,
                                         bias=eps_t[:], scale=1.0)
                    inv = stats.tile([128, 1], FP32, name="inv", tag="inv")
                    nc.vector.reciprocal(out=inv[:], in_=std[:])
                    t2 = tnat_pool.tile([128, D], BF16, name="t2", tag="tn")
                    nc.vector.tensor_scalar(out=t2[:], in0=r4[:, j, :],
                                            scalar1=mv[:, 0:1], scalar2=inv[:],
                                            op0=ALU.subtract, op1=ALU.mult)
                    t2s.append(t2)
                return r4, t2s

            def ffn_ln_tr(t2s):
                t2T_all = ffn_pool.tile([128, 4, 512], wdt, name="t2Tall",
                                        tag="t2Tall")
                for j in range(4):
                    pst = psT.tile([128, 512], BF16, name="psT", tag="psT")
                    for c in range(4):
                        nc.tensor.transpose(pst[:, c * 128:(c + 1) * 128],
                                            t2s[j][:, c * 128:(c + 1) * 128],
                                            ident[:])
                    dst = t2T_all[:, :, j * 128:(j + 1) * 128]
                    if j % 2 == 0:
                        cp(dst, pst[:].rearrange("p (c s) -> p c s", c=4))
                    else:
                        nc.scalar.copy(
                            out=dst, in_=pst[:].rearrange("p (c s) -> p c s", c=4))
                return t2T_all

            def ffn_chunk_ln(ct):
                r4, t2s = ffn_ln_vec(ct)
                return r4, ffn_ln_tr(t2s)

            def ffn_chunk_mm(ct, r4, t2T_all):
                h_all = h_pool.tile([128, 16, 512], wdt, name="hall", tag="hall")
                for mt in range(16):
                    ps = psA.tile([128, 512], FP32, name="psA", tag="psA")
                    if ffn_fp8:
                        for d2 in range(2):
                            nc.tensor.matmul(
                                ps[:],
                                w1v[:, 2 * d2:2 * d2 + 2, mt * 128:(mt + 1) * 128],
                                t2T_all[:, 2 * d2:2 * d2 + 2, :],
                                start=(d2 == 0), stop=(d2 == 1), perf_mode=DR,
                            )
                    else:
                        for d in range(4):
                            nc.tensor.matmul(
                                ps[:], w1v[:, d, mt * 128:(mt + 1) * 128],
                                t2T_all[:, d, :],
                                start=(d == 0), stop=(d == 3),
                            )
                    nc.scalar.activation(out=h_all[:, mt, :], in_=ps[:],
                                         func=AF.Gelu_apprx_tanh,
                                         bias=b1_sb[:, mt:mt + 1],
                                         scale=(1.0 / W8 if ffn_fp8 else 1.0))
                for j in range(4):
                    ps2 = psA.tile([128, 512], FP32, name="psA", tag="psA")
                    if ffn_fp8:
                        for m2 in range(8):
                            nc.tensor.matmul(
                                ps2[:],
                                h_all[:, 2 * m2:2 * m2 + 2, j * 128:(j + 1) * 128],
                                w2v[:, 2 * m2:2 * m2 + 2, :],
                                start=(m2 == 0), stop=(m2 == 7), perf_mode=DR,
                            )
                    else:
                        for mt in range(16):
                            nc.tensor.matmul(
                                ps2[:], h_all[:, mt, j * 128:(j + 1) * 128],
                                w2v[:, mt, :],
                                start=(mt == 0), stop=(mt == 15),
                            )
                    y1 = ostream.tile([128, D], FP32, name="y1", tag="y1")
                    if ffn_fp8:
                        nc.vector.scalar_tensor_tensor(
                            out=y1[:], in0=ps2[:], scalar=1.0 / W8,
                            in1=b2_rep[:], op0=ALU.mult, op1=ALU.add)
                    else:
                        nc.vector.tensor_tensor(out=y1[:], in0=ps2[:],
                                                in1=b2_rep[:], op=ALU.add)
                    nc.vector.tensor_tensor(out=y1[:], in0=y1[:],
                                            in1=r4[:, j, :], op=ALU.add)
                    nc.sync.dma_start(
                        out=out_d.rearrange("(a p) d -> p a d", p=128)[:, ct * 4 + j, :],
                        in_=y1[:])

            # ---------------- main schedule ----------------
            def emit_attn_group(c, half, ffn_ct):
                blk0 = half * 32 + 8 * c
                qd = build_qd_grp(blk0)
                vp = build_gathers(blk0 // 8)
                with nc.named_scope(f"attn{c}{half}"):
                    for i in range(blk0, blk0 + 8):
                        for hp in range(2):
                            if i in (0, NB - 1):
                                flush_prev()
                                emit_global(i, hp, qd_slice(qd, blk0, i, hp))
                            else:
                                cur = ((i, hp, vp),
                                       attn_stage1(i, hp, qd_slice(qd, blk0, i, hp)))
                                flush_prev()
                                prev[0] = cur
                    flush_prev()
                ffn_st = None
                if ffn_ct is not None:
                    with nc.named_scope(f"ffn{ffn_ct}a"):
                        ffn_st = ffn_ln_vec(ffn_ct)
                with nc.named_scope(f"oproj{c}{half}"):
                    for i in range(blk0, blk0 + 8):
                        emit_oT(i)
                    emit_outproj4(16 * half + 4 * c)
                    emit_outproj4(16 * half + 4 * c + 2)
                if ffn_ct is not None:
                    r4, t2s = ffn_st
                    with nc.named_scope(f"ffn{ffn_ct}b"):
                        ffn_chunk_mm(ffn_ct, r4, ffn_ln_tr(t2s))

            rs_issued = [0]
            next_ffn = [0]
            rs_hg = {}
            hg = [0]

            def emit_ffn_next():
                with nc.named_scope(f"ffn{next_ffn[0]}"):
                    ffn_chunk_mm(next_ffn[0], *ffn_chunk_ln(next_ffn[0]))
                next_ffn[0] += 1

            def ffn_ready(hg_now):
                # ffn k once rs k issued >= 2 half-groups (~50us) ago:
                # the CC takes ~20us queue + ~18us to run, and an ffn
                # emitted too early stalls every later vector op behind
                # its rs_out wait (engine streams are in-order)
                return (next_ffn[0] < 4 and next_ffn[0] < rs_issued[0]
                        and hg_now > rs_hg[next_ffn[0]] + (ffn_slack - 1))

            for c in range(4):
                for half in range(2):
                    ct = next_ffn[0] if ffn_ready(hg[0] + 1) else None
                    emit_attn_group(c, half, ct)
                    if ct is not None:
                        next_ffn[0] += 1
                    hg[0] += 1
                    while ffn_ready(hg[0]):
                        emit_ffn_next()
                for jr in range(rs_nchunks // 4):
                    emit_rs(c * (rs_nchunks // 4) + jr)
                rs_issued[0] = c + 1
                rs_hg[c] = hg[0]
            while next_ffn[0] < 4:
                emit_ffn_next()

    nc.compile()
    return nc


_NC_CACHE = None


def _get_nc():
    global _NC_CACHE
    if _NC_CACHE is None:
        _NC_CACHE = build_kernel()
    return _NC_CACHE


def prepare_in_maps(inputs, ffn_fp8=False):
    inp = {k: np.asarray(v, dtype=np.float32) for k, v in inputs.items()}
    x = inp["inputs"]                    # [B, S, D]
    s1, bb1 = inp["ln1_scale"], inp["ln1_bias"]
    s2, bb2 = inp["ln2_scale"], inp["ln2_bias"]
    wq, wk, wv = inp["wq"], inp["wk"], inp["wv"]          # [D, H, DH]
    wo = inp["wo"]                                        # [H, DH, D]
    w1, b1, w2, b2 = inp["w1"], inp["b1"], inp["w2"], inp["b2"]

    scale = 1.0 / np.sqrt(np.float32(DH))
    bf = ml_dtypes.bfloat16
    f8 = ml_dtypes.float8_e4m3

    def to8(a):
        return np.clip(a * W8, -240.0, 240.0).astype(f8)

    in_maps = []
    for p in range(8):
        b = p // 2
        j = p % 2
        hs = slice(4 * j, 4 * j + 4)
        wq_l = wq[:, hs, :].reshape(D, F)
        wk_l = wk[:, hs, :].reshape(D, F)
        wv_l = wv[:, hs, :].reshape(D, F)
        wq_eff = (s1[:, None] * wq_l * scale).astype(bf)
        wk_eff = (s1[:, None] * wk_l).astype(bf)
        wv_eff = (s1[:, None] * wv_l).astype(bf)
        qb = (bb1 @ wq_l * scale).astype(np.float32)
        kb = (bb1 @ wk_l).astype(np.float32)
        wo_l = wo[hs].reshape(F, D).astype(bf)
        w1_sc = s2[:, None] * w1
        b1_eff = (b1 + bb2 @ w1).astype(np.float32)
        # v bias folded into the residual input: o_final = (sum p v)/r + vb,
        # and vb @ wo (all heads) is a constant per-token row added here
        vb_full = bb1 @ wv.reshape(D, H * DH)           # [H*DH]
        vbwo = vb_full @ wo.reshape(H * DH, D)          # [D]
        in_maps.append({
            "x_full": np.ascontiguousarray(x[b]).astype(bf),
            "x_own": (np.ascontiguousarray(x[b, j * TH:(j + 1) * TH]) + vbwo).astype(bf),
            "wq": wq_eff, "wk": wk_eff, "wv": wv_eff, "wo": wo_l,
            "w1": to8(w1_sc) if ffn_fp8 else w1_sc.astype(bf),
            "w2": to8(w2) if ffn_fp8 else w2.astype(bf),
            "qb": qb, "kb": kb,
            "b1": b1_eff, "b2": b2.astype(np.float32),
        })
    return in_maps


def kernel(**inputs):
    in_maps = prepare_in_maps(inputs)
    nc = _get_nc()
    try:
        res = bass_utils.run_bass_kernel_spmd(nc, in_maps, core_ids=list(range(8)))
    except Exception:
        # transient NRT_EXEC_UNIT_UNRECOVERABLE wedges recover on retry
        import time as _time
        _time.sleep(2.0)
        res = bass_utils.run_bass_kernel_spmd(nc, in_maps, core_ids=list(range(8)))
    out = np.zeros((B, S, D), np.float32)
    for p in range(8):
        b, j = p // 2, p % 2
        out[b, j * TH:(j + 1) * TH] = res.results[p]["out"]
    return out


if __name__ == "__main__":
    nc = _get_nc()
    print("built ok")
